# revision 4
# baseline (speedup 1.0000x reference)
"""Causal self-attention (B=4, T=2048, C=2048, H=16) on 8 trn2 NeuronCores.

Sharding: tensor-parallel over heads (2 heads/core). Each core computes the
QKV projection for its head shard (q,k produced transposed for the scores
matmul, v produced in normal layout for attn@v), applies rope fused into the
PSUM->SBUF drain, runs causal attention without max-subtraction (scores are
O(5), exp is fp32-safe), and produces yT = (attn @ v)^T per head. An
AllToAll re-shards Y^T from head-column-sharded to token-row-sharded, after
which each core row-shards the output projection against the full Wproj and
returns its 1024-row chunk of the output. All matmuls use fp32r (single-pass
fp32, ~1e-4 relative error).
"""

import os
import sys

os.environ.setdefault("JAX_PLATFORMS", "axon")

import numpy as np

B, T, C = 4, 2048, 2048
H = 16
HD = 128
N_CORES = 8
HL = H // N_CORES  # heads per core = 2
CL = HL * HD  # per-core head columns = 256
TQ = 512  # Tq chunk for scores
NKT = T // 128  # 16 k-tiles of 128 along T
KC = C // 128  # 16 k-tiles along C
ROWS = B * T // N_CORES  # output rows per core = 1024


def _install_ntff_shim():
    """The agent image's antenv lacks axon_hooks; provide it so
    run_bass_kernel_spmd(trace=True) can reach the NTFF profiler."""
    import types, contextlib, ctypes

    try:
        from antenv.axon_hooks import get_axon_ntff_profile_hook  # noqa

        return
    except ImportError:
        pass

    so_path = "/opt/axon/libaxon_pjrt.so"
    try:
        lib = ctypes.CDLL(so_path)
    except OSError:
        lib = None
    if lib is None or not hasattr(lib, "axon_start_nrt_profile"):
        hook = None
    else:
        lib.axon_start_nrt_profile.argtypes = [
            ctypes.POINTER(ctypes.c_int64),
            ctypes.c_size_t,
        ]
        lib.axon_start_nrt_profile.restype = ctypes.c_int64
        lib.axon_stop_nrt_profile.argtypes = [ctypes.c_char_p]
        lib.axon_stop_nrt_profile.restype = ctypes.c_int64

        @contextlib.contextmanager
        def hook(output_dir, device_ids):
            import jax

            jax.devices()
            if device_ids:
                ids = (ctypes.c_int64 * len(device_ids))(*device_ids)
                rc = lib.axon_start_nrt_profile(ids, len(device_ids))
            else:
                rc = lib.axon_start_nrt_profile(None, 0)
            if rc != 0:
                raise RuntimeError(f"axon_start_nrt_profile rc={rc}")
            try:
                yield
            finally:
                n = lib.axon_stop_nrt_profile(str(output_dir).encode())
                if n <= 0:
                    print(f"ntff profile: rc={n} (no files) dir={output_dir}")

    import antenv

    mod = types.ModuleType("antenv.axon_hooks")
    _state = {"hook": hook}
    mod.set_axon_ntff_profile_hook = lambda h: _state.__setitem__("hook", h)
    mod.get_axon_ntff_profile_hook = lambda: _state["hook"]
    sys.modules["antenv.axon_hooks"] = mod
    antenv.axon_hooks = mod


def build_program():
    import concourse.bass as bass
    import concourse.mybir as mybir
    import concourse.tile as tile
    from concourse import bacc
    from contextlib import ExitStack

    f32 = mybir.dt.float32
    f32r = mybir.dt.float32r
    Exp = mybir.ActivationFunctionType.Exp

    nc = bacc.Bacc("TRN2", target_bir_lowering=False, debug=False, num_devices=N_CORES)

    xT = nc.dram_tensor("xT", [B, C, T], f32r, kind="ExternalInput")
    wqk = nc.dram_tensor("wqk", [C, 4 * HD], f32r, kind="ExternalInput")
    wv = nc.dram_tensor("wv", [C, CL], f32r, kind="ExternalInput")
    wproj = nc.dram_tensor("wproj", [C, C], f32r, kind="ExternalInput")
    cosd = nc.dram_tensor("cos", [HD // 2, T], f32, kind="ExternalInput")
    sind = nc.dram_tensor("sin", [HD // 2, T], f32, kind="ExternalInput")
    out = nc.dram_tensor("out", [ROWS, C], f32, kind="ExternalOutput")

    a2a_in = nc.dram_tensor("a2a_in", [N_CORES, CL, ROWS], f32)
    a2a_out = nc.dram_tensor("a2a_out", [N_CORES, CL, ROWS], f32)

    wqk_t = wqk[:, :].rearrange("(ko p) m -> p ko m", p=128)  # [128, KC, 512]
    wv_t = wv[:, :].rearrange("(ko p) m -> p ko m", p=128)  # [128, KC, 256]
    wproj_t = wproj[:, :].rearrange("(ko p) n -> p ko n", p=128)  # [128, KC, 2048]
    # a2a_out rows (src_core, l) flatten to the global Y column index; view as
    # [p, kt, t] k-tiles for the proj lhsT. Bits are fp32r-rounded already.
    yt_t = (
        a2a_out[:, :, :]
        .rearrange("s (lh p) t -> p (s lh) t", p=128)
        .bitcast(f32r)
    )  # [128, 16, 1024]

    with tile.TileContext(nc) as tc:
        with ExitStack() as top:
            const = top.enter_context(tc.tile_pool(name="const", bufs=1))
            wpool = top.enter_context(tc.tile_pool(name="weights", bufs=1))

            # --- constants ---
            ones_col_f = const.tile([128, 1], f32, tag="ones_col_f")
            nc.vector.memset(ones_col_f[:], 1.0)
            ones_col = const.tile([128, 1], f32r, tag="ones_col")
            nc.vector.tensor_copy(ones_col[:], ones_col_f[:])
            ones_row_f = const.tile([1, 128], f32, tag="ones_row_f")
            nc.vector.memset(ones_row_f[:], 1.0)
            ones_row = const.tile([1, 128], f32r, tag="ones_row")
            nc.vector.tensor_copy(ones_row[:], ones_row_f[:])

            # additive causal masks for the 4 diagonal positions of a Tq chunk:
            # masks[:, d, y] = 0 where y >= x + 128*d else -1e30
            masks = const.tile([128, 4, TQ], f32, tag="masks")
            for d in range(4):
                nc.gpsimd.memset(masks[:, d, :], 0.0)
                nc.gpsimd.affine_select(
                    out=masks[:, d, :],
                    in_=masks[:, d, :],
                    compare_op=mybir.AluOpType.is_ge,
                    fill=-1e30,
                    base=-128 * d,
                    pattern=[[1, TQ]],
                    channel_multiplier=-1,
                )

            cos_sb = const.tile([64, T], f32, tag="cos")
            nc.sync.dma_start(cos_sb[:], cosd[:, :])
            sin_sb = const.tile([64, T], f32, tag="sin")
            nc.sync.dma_start(sin_sb[:], sind[:, :])

            # --- weights resident in SBUF ---
            wqk_sb = wpool.tile([128, KC, 4 * HD], f32r, tag="wqk")
            nc.sync.dma_start(wqk_sb[:], wqk_t)
            wv_sb = wpool.tile([128, KC, CL], f32r, tag="wv")
            nc.sync.dma_start(wv_sb[:], wv_t)

            with ExitStack() as mid:
                qk_pool = mid.enter_context(tc.tile_pool(name="qkT", bufs=1))
                v_pool = mid.enter_context(tc.tile_pool(name="vsb", bufs=1))
                xk_pool = mid.enter_context(tc.tile_pool(name="xk", bufs=20))
                rtmp = mid.enter_context(tc.tile_pool(name="rtmp", bufs=2))
                apool = mid.enter_context(tc.tile_pool(name="apool", bufs=3))
                spool = mid.enter_context(tc.tile_pool(name="spool", bufs=2))
                ps2 = mid.enter_context(tc.tile_pool(name="ps2", bufs=2, space="PSUM"))
                ps1 = mid.enter_context(tc.tile_pool(name="ps1", bufs=1, space="PSUM"))

                for b in range(B):
                    # ---------- QKV projection for batch b ----------
                    # qkT [128, 4, T]: rows m=0,1 -> qT heads 0,1 (rope+scaled),
                    # m=2,3 -> kT heads 0,1 (rope). v_sb [128, NKT, CL].
                    qkT = qk_pool.tile([128, 4, T], f32r, tag="qkT")
                    v_sb = v_pool.tile([128, NKT, CL], f32r, tag="v")

                    for n in range(T // TQ):
                        xk = [
                            xk_pool.tile([128, TQ], f32r, tag="xk", name=f"xk{k}")
                            for k in range(KC)
                        ]
                        for k in range(KC):
                            nc.sync.dma_start(
                                xk[k][:],
                                xT[b, 128 * k : 128 * (k + 1), TQ * n : TQ * (n + 1)],
                            )
                        for m in range(4):
                            qk_ps = ps2.tile([128, TQ], f32, tag="qk")
                            for k in range(KC):
                                nc.tensor.matmul(
                                    qk_ps[:],
                                    wqk_sb[:, k, 128 * m : 128 * (m + 1)],
                                    xk[k][:],
                                    start=(k == 0),
                                    stop=(k == KC - 1),
                                )
                            # rope on the PSUM->SBUF drain
                            cos_t = cos_sb[:, TQ * n : TQ * (n + 1)]
                            sin_t = sin_sb[:, TQ * n : TQ * (n + 1)]
                            t0 = rtmp.tile([64, TQ], f32, tag="t0")
                            t1 = rtmp.tile([64, TQ], f32, tag="t1")
                            nc.vector.tensor_mul(t0[:], qk_ps[0:64, :], cos_t)
                            nc.vector.tensor_mul(t1[:], qk_ps[64:128, :], sin_t)
                            nc.vector.tensor_sub(
                                qkT[0:64, m, TQ * n : TQ * (n + 1)], t0[:], t1[:]
                            )
                            t2 = rtmp.tile([64, TQ], f32, tag="t2")
                            t3 = rtmp.tile([64, TQ], f32, tag="t3")
                            nc.vector.tensor_mul(t2[:], qk_ps[64:128, :], cos_t)
                            nc.vector.tensor_mul(t3[:], qk_ps[0:64, :], sin_t)
                            nc.vector.tensor_add(
                                qkT[64:128, m, TQ * n : TQ * (n + 1)], t2[:], t3[:]
                            )
                        for m2 in range(4):
                            v_ps = ps1.tile([128, CL], f32, tag="v")
                            for k in range(KC):
                                nc.tensor.matmul(
                                    v_ps[:],
                                    xk[k][:, 128 * m2 : 128 * (m2 + 1)],
                                    wv_sb[:, k, :],
                                    start=(k == 0),
                                    stop=(k == KC - 1),
                                )
                            nc.scalar.copy(v_sb[:, 4 * n + m2, :], v_ps[:])

                    # ---------- attention for batch b ----------
                    for h in range(HL):
                        for j in range(T // TQ):
                            ntk = 4 * j + 4  # causal: k-tiles 0..4j+3
                            yT_ps = ps1.tile([128, TQ], f32, tag="yT")
                            asum = spool.tile([128, TQ], f32r, tag="asum")
                            for i in range(ntk):
                                sT_ps = ps2.tile([128, TQ], f32, tag="sT")
                                nc.tensor.matmul(
                                    sT_ps[:],
                                    qkT[:, 2 + h, 128 * i : 128 * (i + 1)],
                                    qkT[:, h, TQ * j : TQ * (j + 1)],
                                    start=True,
                                    stop=True,
                                )
                                d = i - 4 * j
                                if d >= 0:
                                    nc.vector.tensor_add(
                                        sT_ps[:], sT_ps[:], masks[:, d, :]
                                    )
                                a_sb = apool.tile([128, TQ], f32r, tag="a")
                                nc.scalar.activation(a_sb[:], sT_ps[:], Exp)
                                if i == 0:
                                    nc.vector.tensor_copy(asum[:], a_sb[:])
                                else:
                                    nc.vector.tensor_add(asum[:], asum[:], a_sb[:])
                                nc.tensor.matmul(
                                    yT_ps[:],
                                    v_sb[:, i, 128 * h : 128 * (h + 1)],
                                    a_sb[:],
                                    start=(i == 0),
                                    stop=(i == ntk - 1),
                                )
                            # softmax denominator + normalization
                            den_ps = ps1.tile([1, TQ], f32, tag="den")
                            nc.tensor.matmul(
                                den_ps[:], ones_col[:], asum[:], start=True, stop=True
                            )
                            den_sb = spool.tile([1, TQ], f32, tag="den_sb")
                            nc.vector.tensor_copy(den_sb[:], den_ps[:])
                            rec_sb = spool.tile([1, TQ], f32r, tag="rec")
                            with nc.allow_low_precision(
                                reason="fp32r reciprocal feeds fp32r matmul; 1e-4 ok"
                            ):
                                nc.vector.reciprocal(rec_sb[:], den_sb[:])
                            bc_ps = ps1.tile([128, TQ], f32, tag="bc")
                            nc.tensor.matmul(
                                bc_ps[:], ones_row[:], rec_sb[:], start=True, stop=True
                            )
                            bc_sb = spool.tile([128, TQ], f32, tag="bc_sb")
                            nc.scalar.copy(bc_sb[:], bc_ps[:])
                            yT_sb = spool.tile([128, TQ], f32r, tag="yT_sb")
                            nc.vector.tensor_mul(yT_sb[:], yT_ps[:], bc_sb[:])
                            dest = 2 * b + (1 if TQ * j >= ROWS else 0)
                            col0 = (TQ * j) % ROWS
                            nc.sync.dma_start(
                                a2a_in[
                                    dest,
                                    128 * h : 128 * (h + 1),
                                    col0 : col0 + TQ,
                                ],
                                yT_sb[:].bitcast(f32),
                            )

            # ---------- all-to-all: head-sharded Y^T -> row-sharded Y^T ----------
            nc.gpsimd.collective_compute(
                "AllToAll",
                mybir.AluOpType.bypass,
                replica_groups=[list(range(N_CORES))],
                ins=[a2a_in[:, :, :]],
                outs=[a2a_out[:, :, :]],
            )

            # ---------- output projection (row-sharded) ----------
            with ExitStack() as pj:
                ypool = pj.enter_context(tc.tile_pool(name="yproj", bufs=1))
                wp_pool = pj.enter_context(tc.tile_pool(name="wpr", bufs=2))
                opool = pj.enter_context(tc.tile_pool(name="osb", bufs=3))
                ps_o = pj.enter_context(tc.tile_pool(name="pso", bufs=4, space="PSUM"))

                y_sb = ypool.tile([128, KC, ROWS], f32r, tag="y")
                nc.sync.dma_start(y_sb[:], yt_t)
                for n in range(C // TQ):
                    wp_sb = wp_pool.tile([128, KC, TQ], f32r, tag="wp")
                    nc.sync.dma_start(wp_sb[:], wproj_t[:, :, TQ * n : TQ * (n + 1)])
                    for m in range(ROWS // 128):
                        o_ps = ps_o.tile([128, TQ], f32, tag="o")
                        for k in range(KC):
                            nc.tensor.matmul(
                                o_ps[:],
                                y_sb[:, k, 128 * m : 128 * (m + 1)],
                                wp_sb[:, k, :],
                                start=(k == 0),
                                stop=(k == KC - 1),
                            )
                        o_sb = opool.tile([128, TQ], f32, tag="o_sb")
                        nc.scalar.copy(o_sb[:], o_ps[:])
                        nc.sync.dma_start(
                            out[128 * m : 128 * (m + 1), TQ * n : TQ * (n + 1)],
                            o_sb[:],
                        )

    nc.compile()
    return nc


_PERM = None


def _prep_inputs(x, rope, Wqkv, Wproj):
    """Host-side sharding/layout prep (numpy only)."""
    global _PERM
    if _PERM is None:
        _PERM = np.concatenate([np.arange(0, HD, 2), np.arange(1, HD, 2)])
    perm = _PERM

    x = np.ascontiguousarray(x, dtype=np.float32)
    xT = np.ascontiguousarray(x.transpose(0, 2, 1))  # [B, C, T]

    cos = np.ascontiguousarray(rope[:, :, 0].T, dtype=np.float32)  # [64, T]
    sin = np.ascontiguousarray(rope[:, :, 1].T, dtype=np.float32)

    Wq = Wqkv[:, 0:C]
    Wk = Wqkv[:, C : 2 * C]
    Wv = Wqkv[:, 2 * C : 3 * C]
    scale = 1.0 / np.sqrt(HD)

    in_maps = []
    for c in range(N_CORES):
        cols = []
        for lh in range(HL):
            h = HL * c + lh
            cols.append(h * HD + perm)
        qcols = np.concatenate(cols)
        wq_c = Wq[:, qcols] * scale
        wk_c = Wk[:, qcols]
        # interleave per head: [q_h0, k_h0? ] -- layout is [q_h0, q_h1, k_h0, k_h1]
        wqk_c = np.ascontiguousarray(
            np.concatenate([wq_c, wk_c], axis=1), dtype=np.float32
        )  # [C, 512]
        wv_c = np.ascontiguousarray(
            Wv[:, HL * HD * c : HL * HD * (c + 1)], dtype=np.float32
        )  # [C, 256]
        in_maps.append(
            {
                "xT": xT,
                "wqk": wqk_c,
                "wv": wv_c,
                "wproj": np.ascontiguousarray(Wproj, dtype=np.float32),
                "cos": cos,
                "sin": sin,
            }
        )
    return in_maps


_NC_CACHE = None


def _get_nc():
    global _NC_CACHE
    if _NC_CACHE is None:
        _NC_CACHE = build_program()
    return _NC_CACHE


def run(x, rope, Wqkv, Wproj, trace=False):
    _install_ntff_shim()
    from concourse.bass_utils import run_bass_kernel_spmd

    nc = _get_nc()
    in_maps = _prep_inputs(x, rope, Wqkv, Wproj)
    res = run_bass_kernel_spmd(nc, in_maps, list(range(N_CORES)), trace=trace)
    chunks = [res.results[c]["out"] for c in range(N_CORES)]
    full = np.concatenate(chunks, axis=0).reshape(B, T, C)
    return full, res


def kernel(x, rope, Wqkv, Wproj):
    out, _ = run(x, rope, Wqkv, Wproj, trace=False)
    return out


if __name__ == "__main__":
    import time

    t0 = time.time()
    nc = build_program()
    ni = sum(len(bb.instructions) for f in nc.m.functions for bb in f.blocks)
    print(f"build ok: {time.time()-t0:.1f}s, {ni} instructions")
